# revision 9
# baseline (speedup 1.0000x reference)
"""Trainium2 Bass kernel for nn_KernelDensityLoss (KDE softmax loss).

Math: the reference's O(B^2*D) pairwise log-prob matrix collapses to
per-class sufficient statistics.  For row i and class c,

  sums[i,c] = sum_{n in c} lp[i,n]
            = -0.5*(M*const + (M*sq[i] + Ssq[c] - 2*x_i.S_c)/var)

with S_c = sum of class-c embeddings [D], Ssq[c] = sum of squared norms,
sq[i] = ||x_i||^2.  The -0.5*const shift is identical for the own-class
(leave-one-out) and other-class branches and cancels in
logsumexp(row) - own, so the kernel only computes

  A[i,c] = M*sq[i] + Ssq[c] - 2*G[i,c]        (G = X @ S^T)
  P[i,c] = -0.5*A[i,c] / (var*m_c)            (m_c = M-1 own class, M else)
  loss   = sum_i relu(logsumexp_c P[i,c] - P[i,own])

Distribution: B=7168 rows sharded 896/core across 8 NeuronCores.  Each
core computes partial class stats with PE matmuls against the one-hot
class matrix (lhsT = x_tile -> stats come out directly with D on the
partition axis, no transposes), AllGather + local sum combines them
(lower latency floor than AllReduce), then each core evaluates its own
896 rows and emits a partial loss scalar; the host sums 8 scalars.
"""

import numpy as np

import concourse.bass as bass
import concourse.bacc as bacc
import concourse.mybir as mybir
import concourse.tile as tile
from concourse.bass_utils import run_bass_kernel_spmd

B = 7168      # total rows
C = 7         # classes
M = 1024      # rows per class
D = 256       # embedding dim
NCORES = 8
R = B // NCORES          # 896 rows per core
T = R // 128             # 7 row-tiles of 128 per core

F32 = mybir.dt.float32
AX = mybir.AxisListType
AF = mybir.ActivationFunctionType
ALU = mybir.AluOpType

# stats layout (free dim of the [128, SW] stats tile):
#   cols 0:7    S half0  (class sums for d in [0,128))
#   cols 7:14   S half1  (class sums for d in [128,256))
#   row0 14:21  Ssq row  (per-class sum of squared norms)
SW = 24


def build_program():
    nc = bacc.Bacc(
        "TRN2",
        target_bir_lowering=False,
        debug=False,
        enable_asserts=True,
        num_devices=NCORES,
    )

    x_d = nc.dram_tensor("x", [R, D], F32, kind="ExternalInput")
    xf_d = nc.dram_tensor("xf", [B, D], F32, kind="ExternalInput")
    xt_d = nc.dram_tensor("xt", [D, R], F32, kind="ExternalInput")
    y_d = nc.dram_tensor("y", [R, C], F32, kind="ExternalInput")
    yf_d = nc.dram_tensor("yf", [B, C], F32, kind="ExternalInput")
    consts_d = nc.dram_tensor("consts", [128, 4], F32, kind="ExternalInput")
    out_d = nc.dram_tensor("loss_part", [1, 1], F32, kind="ExternalOutput")
    TF = B // 128  # 56 tiles over the full batch

    with tile.TileContext(nc) as tc:
        with (
            tc.tile_pool(name="persist", bufs=1) as pp,
            tc.tile_pool(name="xtiles", bufs=4) as px,
            tc.tile_pool(name="scratch", bufs=2) as ps,
            tc.tile_pool(name="chunk", bufs=2) as pc,
            tc.tile_pool(name="psum_stat", bufs=1, space="PSUM") as qstat,
            tc.tile_pool(name="psum_p", bufs=2, space="PSUM") as qp,
            tc.tile_pool(name="dram", bufs=1, space="DRAM") as pd,
        ):
            # ---- persistent tiles ----
            xt0 = pp.tile([128, R], F32, tag="xt0")      # d in [0,128)
            xt1 = pp.tile([128, R], F32, tag="xt1")      # d in [128,256)
            ytile = pp.tile([128, T, C], F32, tag="y")   # own-shard mask [p, t, c]
            yftile = pp.tile([128, TF, C], F32, tag="yf")  # full one-hot [p, g, c]
            consts = pp.tile([128, 4], F32, tag="consts")
            sq = pp.tile([128, T], F32, tag="sq")        # own-shard ||x||^2
            sqf = pp.tile([128, TF], F32, tag="sqf")     # full-batch ||x||^2
            b_oth = pp.tile([128, T], F32, tag="b_oth")
            stats = pp.tile([128, SW], F32, tag="stats")
            shsc = pp.tile([128, 2 * C], F32, tag="shsc")
            accL = pp.tile([128, T], F32, tag="accL")
            accT = pp.tile([128, 1], F32, tag="accT")
            ones_row = pp.tile([1, 128], F32, tag="ones_row")
            ones_col = pp.tile([128, 1], F32, tag="ones_col")
            out_s = pp.tile([1, 1], F32, tag="out_s")

            ph0 = qstat.tile([128, C], F32, tag="ph0")
            ph1 = qstat.tile([128, C], F32, tag="ph1")
            pssq = qstat.tile([1, C], F32, tag="pssq")
            ploss = qstat.tile([1, 1], F32, tag="ploss")

            # ---- loads ----
            nc.sync.dma_start(out=consts[:], in_=consts_d[:, :])
            nc.sync.dma_start(
                out=ytile[:],
                in_=y_d.ap().rearrange("(t p) c -> p t c", p=128),
            )
            nc.sync.dma_start(
                out=yftile[:],
                in_=yf_d.ap().rearrange("(t p) c -> p t c", p=128),
            )
            for h in range(T):
                lo, hi = h * 128, (h + 1) * 128
                nc.sync.dma_start(out=xt0[:, lo:hi], in_=xt_d[0:128, lo:hi])
                nc.sync.dma_start(out=xt1[:, lo:hi], in_=xt_d[128:256, lo:hi])

            nc.vector.memset(ones_row[:], 1.0)
            nc.vector.memset(ones_col[:], 1.0)
            nc.vector.memset(stats[:], 0.0)

            # ---- own-shard row norms (for the per-row bias) ----
            for t in range(T):
                x_t = px.tile([128, D], F32, tag="x_t")
                nc.sync.dma_start(out=x_t[:], in_=x_d[t * 128:(t + 1) * 128, :])
                # (tensor_tensor_reduce faults the exec unit on this runtime)
                xsq = ps.tile([128, D], F32, tag="xsq")
                nc.scalar.activation(xsq[:], x_t[:], AF.Square,
                                     bias=0.0, scale=1.0, accum_out=sq[:, t:t + 1])
            # b_oth = (-0.5/var)*sq
            nc.scalar.activation(b_oth[:], sq[:], AF.Copy, bias=0.0, scale=consts[:, 2:3])

            # ---- full-batch class stats, replicated on every core ----
            # (an 8-core AllGather costs ~56us on this runtime — recomputing
            # from a full replicated load is far cheaper and fully overlaps DMA)
            for g in range(TF):
                xg = px.tile([128, D], F32, tag="xg")
                nc.sync.dma_start(out=xg[:], in_=xf_d[g * 128:(g + 1) * 128, :])

                if g % 2 == 0:
                    xsg = ps.tile([128, D], F32, tag="xsg")
                    nc.scalar.activation(xsg[:], xg[:], AF.Square,
                                         bias=0.0, scale=1.0, accum_out=sqf[:, g:g + 1])
                else:
                    xsg = ps.tile([128, D], F32, tag="xsg2")
                    nc.vector.tensor_mul(xsg[:], xg[:], xg[:])
                    nc.vector.reduce_sum(sqf[:, g:g + 1], xsg[:], axis=AX.X)

                y_g = yftile[:, g, :]
                st = (g == 0)
                sp = (g == TF - 1)
                nc.tensor.matmul(ph0[:], lhsT=xg[:, 0:128], rhs=y_g, start=st, stop=sp)
                nc.tensor.matmul(ph1[:], lhsT=xg[:, 128:256], rhs=y_g, start=st, stop=sp)
                nc.tensor.matmul(pssq[:], lhsT=sqf[:, g:g + 1], rhs=y_g, start=st, stop=sp)

            # ---- combine: stats tile directly from PSUM ----
            nc.scalar.copy(stats[:, 0:C], ph0[:])
            nc.scalar.copy(stats[:, C:2 * C], ph1[:])
            nc.scalar.copy(stats[0:1, 2 * C:3 * C], pssq[:])
            # shsc = -2 * S (both halves)
            nc.vector.tensor_scalar_mul(shsc[:], stats[:, 0:2 * C], -2.0)

            # ---- phase 3: per-row loss ----
            for u in range(T):
                lo, hi = u * 128, (u + 1) * 128
                pP = qp.tile([128, C], F32, tag="pP")
                # pP = -2*G + Ssq  (c-dependent part of A)
                nc.tensor.matmul(pP[:], lhsT=xt0[:, lo:hi], rhs=shsc[:, 0:C],
                                 start=True, stop=False)
                nc.tensor.matmul(pP[:], lhsT=xt1[:, lo:hi], rhs=shsc[:, C:2 * C],
                                 start=False, stop=False)
                nc.tensor.matmul(pP[:], lhsT=ones_row[:], rhs=stats[0:1, 2 * C:3 * C],
                                 start=False, stop=True)

                # P_oth for all 7 columns; the own-class (leave-one-out)
                # value is an exact rescale: P_own = P_oth * M/(M-1), so the
                # select fuses into one multiply-add against the one-hot mask.
                p_oth = pc.tile([128, C], F32, tag="p_oth")
                nc.scalar.activation(p_oth[:], pP[:], AF.Identity,
                                     bias=b_oth[:, u:u + 1], scale=consts[:, 0:1])

                mask_u = ytile[:, u, :]
                # scr7raw = mask * p_oth  (only own column nonzero)
                scr7 = pc.tile([128, C], F32, tag="scr7")
                nc.vector.tensor_tensor(scr7[:], p_oth[:], mask_u, op=ALU.mult)
                # own value (pre-LOO): P_oth[own] = row-sum of scr7raw
                own_raw = pc.tile([128, 1], F32, tag="own_raw")
                nc.vector.reduce_sum(own_raw[:], scr7[:], axis=AX.X)
                # p_fin: own column scaled by M/(M-1) (the exact LOO value)
                sc2 = pc.tile([128, C], F32, tag="sc2")
                nc.vector.tensor_scalar_mul(sc2[:], scr7[:], 1.0 / (M - 1))
                p_fin = pc.tile([128, C], F32, tag="p_fin")
                nc.vector.tensor_add(p_fin[:], p_oth[:], sc2[:])

                nmx = pc.tile([128, 1], F32, tag="nmx")
                nc.vector.tensor_reduce(
                    out=nmx[:], in_=p_fin[:], axis=AX.X, op=ALU.max, negate=True
                )

                ex = pc.tile([128, C], F32, tag="ex")
                se = pc.tile([128, 1], F32, tag="se")
                nc.scalar.activation(ex[:], p_fin[:], AF.Exp,
                                     bias=nmx[:], scale=1.0, accum_out=se[:])
                lnse = pc.tile([128, 1], F32, tag="lnse")
                nc.scalar.activation(lnse[:], se[:], AF.Ln)

                # L = (lnse - nmx) - M/(M-1)*own_raw ; accL[:,u] = relu(L)
                s1 = pc.tile([128, 1], F32, tag="s1")
                nc.vector.tensor_sub(s1[:], lnse[:], nmx[:])
                ot = pc.tile([128, 1], F32, tag="ot")
                nc.vector.tensor_scalar_mul(ot[:], own_raw[:], -float(M) / (M - 1))
                l_u = pc.tile([128, 1], F32, tag="l_u")
                nc.vector.tensor_add(l_u[:], s1[:], ot[:])
                nc.vector.tensor_scalar_max(accL[:, u:u + 1], l_u[:], 0.0)

            # ---- reduce to scalar ----
            nc.vector.reduce_sum(accT[:], accL[:], axis=AX.X)
            nc.tensor.matmul(ploss[:], lhsT=accT[:], rhs=ones_col[:],
                             start=True, stop=True)
            nc.scalar.copy(out_s[:], ploss[:])
            nc.sync.dma_start(out=out_d[:, :], in_=out_s[:])

    nc.compile()
    return nc


_NC_CACHE = None


def _get_nc():
    global _NC_CACHE
    if _NC_CACHE is None:
        _NC_CACHE = build_program()
    return _NC_CACHE


def make_in_maps(embeddings, variance):
    X = np.ascontiguousarray(np.asarray(embeddings, dtype=np.float32))
    assert X.shape == (B, D), X.shape
    var = float(np.asarray(variance))

    labels = np.repeat(np.arange(C), M)  # reference ignores `target`
    Yfull = np.zeros((B, C), np.float32)
    Yfull[np.arange(B), labels] = 1.0

    consts = np.zeros((128, 4), np.float32)
    consts[:, 0] = -0.5 / (var * M)
    consts[:, 1] = -0.5 / (var * (M - 1))
    consts[:, 2] = -0.5 / var
    consts[:, 3] = -0.5 * M / (var * (M - 1))

    in_maps = []
    for k in range(NCORES):
        s = slice(k * R, (k + 1) * R)
        in_maps.append({
            "x": X[s],
            "xf": X,
            "xt": np.ascontiguousarray(X[s].T),
            "y": np.ascontiguousarray(Yfull[s]),
            "yf": Yfull,
            "consts": consts,
        })
    return in_maps


def kernel(embeddings, target, variance):
    del target  # labels are balanced & class-sorted by construction (as in reference)
    nc = _get_nc()
    in_maps = make_in_maps(embeddings, variance)
    res = run_bass_kernel_spmd(nc, in_maps, list(range(NCORES)))
    total = 0.0
    for k in range(NCORES):
        total += float(res.results[k]["loss_part"][0, 0])
    return np.float32(total)
